# revision 34
# baseline (speedup 1.0000x reference)
"""AttentionHead kernel for 8x TRN2 NeuronCores (Bass/Tile on Bacc).

Problem: single-head attention, S=4096, B=4, D=128, C=K=V=64, f32 inputs,
int32 {0,1} mask [1, S, S] applied before softmax (mask==0 -> -inf).

Sharding: queries sharded across 8 cores (512 q/core, all 4 batches per
core).

Math (per core, per batch), all PE contractions on partitions:
  qq = W2 q + b2 where W2 = wk^T wq (host-folded; per-q bias bk.q is
       softmax-invariant and dropped), cast fp8e4.
  scores^T[s, q] = sum_d key8[d,s] qq8[d,q]  +  maskbias[s, q]
       computed as ONE fp8 DoubleRow matmul per 128-s-tile chunk:
       lhsT [128, 2, 128] = [key8_tile | identity]   (host-interleaved)
       rhs  [128, 2, 512] = [qq8 | mask8_chunk]      (custom-stride AP over
            one SBUF arena [qq_b3..qq_b0 | mask(16K)]; the identity j-slot
            delivers maskbias = -240*(1-mask) exactly for free. The arena
            order makes each AP's bounding footprint cover only
            already-written qq slots and mask chunks <= m, so scores gate
            progressively on mask DMA arrival with no WAR hazards.)
  alpha = exp(scores/8)  (ACT, [128, 1536] groups, writes fp8 directly;
       masked entries exp(~-28) underflow to exactly 0)
  v_ext[s, c'] = fp8(value_tile^T wv), c'=64 column = 1 (memset), built
       directly in [s, c'] orientation (no transposes); bias bv deferred.
  comb[c', q] += v_ext_pair^T alpha_pair   (fp8 DoubleRow, K=256: two
       s-tiles per matmul; row 64 accumulates the softmax denominator)
  out[q, :] = comb[0:64]/comb[64] + bv     (PE transpose + fused
       affine_then_add: *recip + bv, bv host-replicated [128, 64])

Perf structure: ACT exp (11 instrs x ~1.55us per batch) is the bottleneck
engine; PE (scores 32 + va2 16 + vproj 32 + qq/epi per batch) runs ~50us
busy with slack, DVE ~15us, DMA ~8.5 MiB/core. Deep software pipeline:
group g scores || g-1 exp || lagged va2 pairs || staggered vproj/qq/epilogue
pending tasks keep every engine fed across batch seams.
"""

import os
import sys

import numpy as np

if "/opt/trn_rl_repo" not in sys.path:
    sys.path.insert(0, "/opt/trn_rl_repo")

S, B, D, C = 4096, 4, 128, 64
NCORES = 8
QS = S // NCORES  # 512 queries per core
QT = QS // 128  # 4 q tiles
ST = S // 128  # 32 s tiles per batch
NG = 11  # exp groups per batch: 10x(3 chunks) + 1x(2 chunks)
NP = ST // 2  # 16 va2 pairs per batch
SLOT = 128  # v_ext slot stride in elements (64 proj + 1 ones + pad;
# LDWEIGHTS DoubleRow requires well-aligned j-plane strides — 68 fails
# the walrus ISA check, 128 is the micro-proven shape)
MASKW = ST * QS  # 16384 arena mask columns
SCALE = 0.125  # 1/sqrt(64)
MASKBIAS = -240.0  # exact in fp8e4m3; exp(scale*(x-240)) == 0 for |x|<~100
ALPHA_FP8 = True  # False: bf16 alpha + non-DR va2 (higher precision)

LAST_RESULT = None
KVER = 50  # bumped per kernel revision: defeats HLO-fingerprint NEFF-cache aliasing


def _install_ntff_hook():
    """The grading/axon image lacks antenv.axon_hooks; recreate it so
    trace=True can capture NTFF profiles. Harmless no-op when unavailable."""
    import types

    try:
        import antenv

        try:
            from antenv import axon_hooks  # noqa: F401

            return
        except ImportError:
            pass
        from trn_agent_boot.trn_boot import _ntff_profile_via_ctypes

        mod = types.ModuleType("antenv.axon_hooks")
        _h = [_ntff_profile_via_ctypes("/opt/axon/libaxon_pjrt.so")]
        mod.get_axon_ntff_profile_hook = lambda: _h[0]
        mod.set_axon_ntff_profile_hook = lambda h: _h.__setitem__(0, h)
        sys.modules["antenv.axon_hooks"] = mod
        antenv.axon_hooks = mod
    except Exception:
        pass


def _gwidth(g):
    """chunks in group g (local index)"""
    return 3 if g < 10 else 2


def _gcol(g):
    """first chunk index of group g"""
    return 3 * g


def _build_nc():
    import concourse.mybir as mybir
    from concourse import bacc
    from concourse.masks import make_identity
    from concourse.tile import TileContext

    f32 = mybir.dt.float32
    bf16 = mybir.dt.bfloat16
    f8 = mybir.dt.float8e4
    AF = mybir.ActivationFunctionType
    DR = mybir.MatmulPerfMode.DoubleRow
    a_dt = f8 if ALPHA_FP8 else bf16

    nc = bacc.Bacc("TRN2")

    # Inputs byte-packed into few large DMA blocks (each dma_start costs
    # ~700ns serial issue time on the SP sequencer):
    #   blk0: consts(656) | qT_b0 bf16(1024) | key8_b0 chunks 0-15 (4096)
    #   blkm: key8_b0 chunks 16-31 (4096) | valT_b0 bf16 (8192)
    #   blk[b-1] (b=1..3): qT(1024) | key8(8192) | valT(8192)
    # key8 slot layout per s-tile: [key_tile fp8 (128) | identity fp8 (128)]
    u8 = mybir.dt.uint8
    blk0_d = nc.dram_tensor("blk0", [128, 5776], u8, kind="ExternalInput")
    blkm_d = nc.dram_tensor("blkm", [128, 12288], u8, kind="ExternalInput")
    blk_d = nc.dram_tensor("blk", [128, 3, 17408], u8, kind="ExternalInput")
    # mask pre-swizzled on host to the arena layout [p, m*512+q]
    mask8_d = nc.dram_tensor("mask8", [128, MASKW], f8, kind="ExternalInput")
    # output layout [p, (b, qt, c)]: one contiguous 1KB-per-partition DMA
    # per batch (the [q, b, c] layout needed 256B descriptors); host
    # unpacks to [QS, B, C]
    out_d = nc.dram_tensor("out", [128, B * QT * C], f32, kind="ExternalOutput")
    # dummy input whose shape encodes the kernel revision: the PJRT-side NEFF
    # cache keys on the HLO signature (not the embedded BIR), so same-shaped
    # kernel revisions would otherwise silently alias to a stale executable.
    nc.dram_tensor("vtag", [KVER], f32, kind="ExternalInput")

    with TileContext(nc) as tc:
        with (
            tc.tile_pool(name="consts", bufs=1) as consts,
            tc.tile_pool(name="big", bufs=1) as big,
            tc.tile_pool(name="pb", bufs=2) as pb,
            tc.tile_pool(name="apool", bufs=2) as apool,
            tc.tile_pool(name="work", bufs=4) as work,
            tc.tile_pool(name="scps", bufs=2, space="PSUM") as scps,
            tc.tile_pool(name="ppps", bufs=1, space="PSUM") as ppps,
            tc.tile_pool(name="accps", bufs=1, space="PSUM") as accps,
        ):
            # ---------------- constants ----------------
            ident_f = consts.tile([128, 128], f32, tag="ident_f")
            make_identity(nc, ident_f[:])

            blk0 = big.tile([128, 5776], u8, tag="blk0")
            nc.sync.dma_start(out=blk0[:], in_=blk0_d[:, :])
            w2T = blk0[:, 0:256].bitcast(bf16)
            wvT = blk0[:, 256:384].bitcast(bf16)
            b2 = blk0[:, 384:388].bitcast(f32)
            bvrep = blk0[:, 400:656].bitcast(f32)

            # arena: [qq_b3 qq_b2 qq_b1 qq_b0 | mask (MASKW)] fp8.
            # qq slots REVERSED and ahead of the mask: scores(b, m)'s AP
            # bounding footprint is then [qq_b .. mask_m], i.e. only already-
            # written qq slots (no WAR on future batches' qq) and only mask
            # chunks <= m (progressive gating on mask DMA arrival).
            AQ = B * QS
            arena = big.tile([128, AQ + MASKW], f8, tag="arena")

            # One dma_start's descriptors spread across all 16 physical
            # queues, but each dma_start costs ~700ns of serial issue time on
            # its engine's sequencer. So: few, large dma_starts, spread across
            # engine sequencers (SP for startup-critical, idle Pool/DVE for
            # bulk prefetch) so issues proceed in parallel.
            def load_mask():
                for j in range(4):
                    nc.sync.dma_start(
                        out=arena[:, AQ + j * 4096 : AQ + (j + 1) * 4096],
                        in_=mask8_d[:, j * 4096 : (j + 1) * 4096],
                    )

            def scores_rhs(b, m):
                """custom AP [128, 2, 512]: j=0 -> qq_b, j=1 -> mask chunk m
                (pairs lhsT slot [key | I])"""
                o = (B - 1 - b) * QS
                base = arena[:, o : o + QS]
                ap = base.unsqueeze(1)
                l = ap.ap
                l[1] = [AQ + m * QS - o, 2]
                ap.ap = l
                return ap

            blk_by_b = {}

            def load_batch(b):
                blk = pb.tile([128, 17408], u8, tag="blk")
                nc.sync.dma_start(out=blk[:], in_=blk_d[:, b - 1, :])
                blk_by_b[b] = blk

            def qt_ap(b):
                if b == 0:
                    return blk0[:, 656:1680].bitcast(bf16)
                return blk_by_b[b][:, 0:1024].bitcast(bf16)

            def key_lhsT(b, m):
                if b == 0:
                    if m < 16:
                        sl = blk0[:, 1680 + m * 256 : 1680 + (m + 1) * 256]
                    else:
                        sl = blkm[:, (m - 16) * 256 : (m - 15) * 256]
                else:
                    sl = blk_by_b[b][:, 1024 + m * 256 : 1024 + (m + 1) * 256]
                return sl.bitcast(f8).rearrange("p (j s) -> p j s", j=2)

            def val_slice(b, st):
                if b == 0:
                    sl = blkm[:, 4096 + st * 256 : 4096 + (st + 1) * 256]
                else:
                    sl = blk_by_b[b][:, 9216 + st * 256 : 9216 + (st + 1) * 256]
                return sl.bitcast(bf16)

            def qq_tasks(b, qT):
                cell = {}

                def qq_mm():
                    qq_ps = ppps.tile([128, QS], f32, tag="pp", name="qq_ps")
                    nc.tensor.matmul(qq_ps[:], w2T, qT, start=True, stop=True)
                    cell["ps"] = qq_ps

                def qq_cp():
                    nc.vector.tensor_scalar_add(
                        out=arena[:, (B - 1 - b) * QS : (B - b) * QS],
                        in0=cell["ps"][:],
                        scalar1=b2,
                    )

                return [(qq_mm, qq_cp)]

            def v_tasks(vb, v_ext):
                """Direct-orientation vproj: out[s, c] tiles, batched copies."""
                pairs = []
                # ones column: c'=64 of each slot
                pairs.append(
                    (
                        lambda: nc.vector.memset(
                            v_ext[:].rearrange("p (t c) -> p t c", c=SLOT)[
                                :, :, C : C + 1
                            ],
                            1.0,
                        ),
                        None,
                    )
                )
                # first group small so its copy lands (in program order)
                # before the first va2 pair enters the PE queue
                bounds = [0, 2, 8, 14, 20, 26, 32]
                for gi in range(len(bounds) - 1):
                    g0, g1 = bounds[gi], bounds[gi + 1]
                    gs = g1 - g0
                    cell = {}
                    for k in range(gs):

                        def vp_mm(k=k, g0=g0, cell=cell, first=(k == 0)):
                            if first:
                                cell["ps"] = ppps.tile(
                                    [128, 7 * C], f32, tag="pp", name="vp_ps"
                                )
                            nc.tensor.matmul(
                                cell["ps"][:, k * C : (k + 1) * C],
                                val_slice(vb, g0 + k),
                                wvT,
                                start=True,
                                stop=True,
                            )

                        pairs.append((vp_mm, None))

                    def vp_cp(g0=g0, gs=gs, cell=cell):
                        nc.vector.tensor_copy(
                            out=v_ext[:, g0 * SLOT : (g0 + gs) * SLOT].rearrange(
                                "p (t c) -> p t c", c=SLOT
                            )[:, :, :C],
                            in_=cell["ps"][:, : gs * C].rearrange(
                                "p (t c) -> p t c", c=C
                            ),
                        )

                    pairs.append((None, vp_cp))
                return pairs

            def stagger(pairs):
                """Each step emits the PREVIOUS task's copy before this task's
                mm so the single-buffer pp ring never stalls the PE queue."""
                steps = []
                prev_cp = [None]

                def mk(mm, pc):
                    def step():
                        if pc is not None:
                            pc()
                        if mm is not None:
                            mm()

                    return step

                for mm, cp in pairs:
                    steps.append(mk(mm, prev_cp[0]))
                    prev_cp[0] = cp
                if prev_cp[0] is not None:
                    steps.append(lambda pc=prev_cp[0]: pc())
                return steps

            def epilogue_tasks(b, acc_ps, last=False):
                cell = {}
                steps = []

                def comb_step():
                    comb = work.tile([C + 1, QS], f32, tag="comb")
                    nc.vector.tensor_copy(out=comb[:], in_=acc_ps[:])
                    fin = work.tile([128, QT * C], f32, tag="fin")
                    cell["comb"] = comb
                    cell["fin"] = fin

                steps.append(comb_step)
                for qt in range(QT):

                    def qt_step(qt=qt):
                        if last:
                            # sc ring is free after the final exp; borrowing
                            # it unserializes the tail epilogue (pp bufs=1)
                            ot_ps = scps.tile(
                                [128, 1536], f32, tag="sc", name="sc"
                            )[:, : C + 1]
                        else:
                            ot_ps = ppps.tile(
                                [128, C + 1], f32, tag="pp", name="ot_ps"
                            )
                        nc.tensor.transpose(
                            ot_ps[:],
                            cell["comb"][:, qt * 128 : (qt + 1) * 128],
                            ident_f[: C + 1, : C + 1],
                        )
                        recip = work.tile([128, 1], f32, tag="recip")
                        nc.vector.reciprocal(recip[:], ot_ps[:, C : C + 1])
                        nc.vector.affine_then_add(
                            out=cell["fin"][:, qt * C : (qt + 1) * C],
                            in0=ot_ps[:, :C],
                            in1=bvrep,
                            scale=recip[:],
                            bias=0.0,
                        )

                    steps.append(qt_step)
                    if qt == 1:

                        def out_half():
                            nc.sync.dma_start(
                                out=out_d[:, b * QT * C : b * QT * C + 2 * C],
                                in_=cell["fin"][:, : 2 * C],
                            )

                        steps.append(out_half)

                def out_step():
                    nc.sync.dma_start(
                        out=out_d[:, b * QT * C + 2 * C : (b + 1) * QT * C],
                        in_=cell["fin"][:, 2 * C :],
                    )

                steps.append(out_step)
                return steps

            # ---------- flat pipeline over all B*NG groups ----------
            # Batch 0 prologue runs INLINE (not via pending): qq + all vproj
            # matmuls execute on PE while the mask arena streams in, instead
            # of scores(0,*) blocking the in-order PE queue on the mask DMAs.
            # batch-0 DMA order: blk0 (consts+qT+key8a, scores-critical),
            # mask (progressive), blkm (key8b+valT; vproj is pending-drained
            # so it never blocks the first scores in the PE queue)
            load_mask()
            blkm = big.tile([128, 12288], u8, tag="blkm")
            nc.sync.dma_start(out=blkm[:], in_=blkm_d[:, :])
            vext_by_b = {}
            alpha_by_b = {}
            for mm, cp in qq_tasks(0, qt_ap(0)):
                mm()
                if cp is not None:
                    cp()
            v_ext0 = pb.tile([128, ST * SLOT], a_dt, tag="v_ext")
            vext_by_b[0] = v_ext0
            pending = stagger(v_tasks(0, v_ext0))

            total = B * NG
            KEXP = 1  # exp lags scores by 1 group
            scs = {}
            acc_by_b = {}
            va_cursor = 0  # global va2 pair index
            exp_done = -1  # last global group exp'd

            def issue_va2(limit_pairs):
                """Issue ready va2 pairs up to global pair index `limit_pairs`
                (exclusive)."""
                nonlocal va_cursor
                while va_cursor < min(limit_pairs, B * NP):
                    v = va_cursor
                    b_v, u = divmod(v, NP)
                    if u == 0:
                        acc_by_b[b_v] = accps.tile(
                            [C + 1, QS], f32, tag="acc", name="acc"
                        )
                    acc = acc_by_b[b_v]
                    v_ext = vext_by_b[b_v]
                    alpha = alpha_by_b[b_v]
                    if ALPHA_FP8:
                        lhsT = v_ext[:, u * 2 * SLOT : (u + 1) * 2 * SLOT].rearrange(
                            "p (j c) -> p j c", c=SLOT
                        )[:, :, : C + 1]
                        rhs = alpha[:, u * 1024 : (u + 1) * 1024].rearrange(
                            "p (j q) -> p j q", j=2
                        )
                        nc.tensor.matmul(
                            acc[:],
                            lhsT,
                            rhs,
                            start=(u == 0),
                            stop=(u == NP - 1),
                            perf_mode=mybir.MatmulPerfMode.DoubleRow,
                        )
                    else:
                        for h in range(2):
                            st = 2 * u + h
                            nc.tensor.matmul(
                                acc[:],
                                v_ext[:, st * SLOT : st * SLOT + C + 1],
                                alpha[:, st * 512 : (st + 1) * 512],
                                start=(st == 0),
                                stop=(st == ST - 1),
                            )
                    va_cursor += 1
                    if u == NP - 1:
                        ep = epilogue_tasks(
                            b_v, acc_by_b.pop(b_v), last=(b_v == B - 1)
                        )
                        pending[:0] = [ep[0]]
                        pending.extend(ep[1:])
                        del vext_by_b[b_v]
                        del alpha_by_b[b_v]

            for gp in range(total + KEXP + 2):
                if gp < total:
                    b_s, g_s = divmod(gp, NG)
                    if g_s == 0:
                        alpha_by_b[b_s] = apool.tile(
                            [128, ST * 512], a_dt, tag="alpha", name="alpha"
                        )
                        if b_s + 1 < B:
                            load_batch(b_s + 1)
                            v_ext_n = pb.tile([128, ST * SLOT], a_dt, tag="v_ext")
                            vext_by_b[b_s + 1] = v_ext_n
                            pending.extend(
                                stagger(
                                    qq_tasks(b_s + 1, qt_ap(b_s + 1))
                                    + v_tasks(b_s + 1, v_ext_n)
                                )
                            )
                    w = _gwidth(g_s)
                    sc = scps.tile([128, 1536], f32, tag="sc", name="sc")
                    for ci in range(w):
                        m = _gcol(g_s) + ci
                        nc.tensor.matmul(
                            sc[:, ci * 512 : (ci + 1) * 512],
                            key_lhsT(b_s, m),
                            scores_rhs(b_s, m),
                            start=True,
                            stop=True,
                            perf_mode=DR,
                        )
                    scs[gp] = (sc, b_s, g_s, w)
                if KEXP <= gp < total + KEXP:
                    v = gp - KEXP
                    sc, b_e, g_e, w = scs.pop(v)
                    alpha = alpha_by_b[b_e]
                    c0 = _gcol(g_e) * 512
                    nc.scalar.activation(
                        out=alpha[:, c0 : c0 + w * 512],
                        in_=sc[:, : w * 512],
                        func=AF.Exp,
                        scale=SCALE,
                    )
                    exp_done = v
                    # va2 pairs fully covered by exp'd groups (one group slack)
                    eb, eg = divmod(exp_done, NG)
                    # within batch eb: chunks done through _gcol(eg)+_gwidth(eg)
                    chunks_done = _gcol(eg) + _gwidth(eg)
                    pairs_done = chunks_done // 2
                    issue_va2(eb * NP + max(0, pairs_done - 2))
                if gp >= total + KEXP:
                    issue_va2(B * NP)
                for thresh in (0, 0, 6, 12, 20):
                    if len(pending) > thresh:
                        pending.pop(0)()
            issue_va2(B * NP)
            while pending:
                pending.pop(0)()

    nc.finalize()
    return nc


_nc_cache = None


def kernel(**inputs):
    global _nc_cache, LAST_RESULT
    _install_ntff_hook()
    import ml_dtypes

    from concourse.bass_utils import run_bass_kernel_spmd

    bf16 = ml_dtypes.bfloat16
    f8 = ml_dtypes.float8_e4m3
    arrs = {k: np.asarray(v) for k, v in inputs.items()}

    keyT = np.ascontiguousarray(
        arrs["key"].astype(np.float32).transpose(2, 1, 0)
    )  # [D, B, S]
    key8 = np.zeros([D, B, ST * 256], dtype=f8)
    eye8 = np.eye(128, dtype=np.float32).astype(f8)
    k8 = keyT.astype(f8)
    for st in range(ST):
        key8[:, :, st * 256 : st * 256 + 128] = k8[:, :, st * 128 : (st + 1) * 128]
        key8[:, :, st * 256 + 128 : (st + 1) * 256] = eye8[:, None, :]

    value = np.ascontiguousarray(
        arrs["value"].astype(np.float32).transpose(2, 1, 0)
    ).astype(bf16)  # [D, B, S]
    query = np.ascontiguousarray(arrs["query"], dtype=np.float32)
    mask = np.ascontiguousarray(arrs["mask"], dtype=np.int32)
    if mask.ndim == 3:
        mask = mask[0]
    maskbias = np.where(mask == 0, np.float32(MASKBIAS), np.float32(0.0))
    mask8_full = maskbias.astype(f8)  # [S(q-axis? no: rows=q of reference), S]

    wk_f = arrs["wk_w"].astype(np.float64)
    wq_f = arrs["wq_w"].astype(np.float64)
    # scores = (wk key).(wq q + bq) = key^T (W2 q + b2); lhsT for the qq
    # matmul is W2^T = wq^T wk
    w2T = np.ascontiguousarray(wq_f.T @ wk_f).astype(np.float32).astype(bf16)
    b2h = np.ascontiguousarray(
        (wk_f.T @ arrs["wq_b"].astype(np.float64)).astype(np.float32)
    ).reshape(D, 1)
    wvT = np.ascontiguousarray(arrs["wv_w"].astype(np.float32).T).astype(bf16)
    bvrep = np.ascontiguousarray(
        np.broadcast_to(arrs["wv_b"].astype(np.float32)[None, :], (128, C))
    ).astype(np.float32)
    cpk = np.zeros([128, 656], np.uint8)
    cpk[:, 0:256] = w2T.view(np.uint8).reshape(128, 256)
    cpk[:, 256:384] = wvT.view(np.uint8).reshape(128, 128)
    cpk[:, 384:388] = b2h.view(np.uint8).reshape(128, 4)
    cpk[:, 400:656] = bvrep.view(np.uint8).reshape(128, 256)

    key8_u8 = key8.view(np.uint8)  # [128, B, 8192]
    val_u8 = value.view(np.uint8)  # [128, B, 8192]

    if _nc_cache is None:
        _nc_cache = _build_nc()
    nc = _nc_cache

    in_maps = []
    for i in range(NCORES):
        q0 = i * QS
        # scores^T[s, q] is masked by mask[q_global, s]; arena layout
        # [p, m*512 + q] = maskbias[q0+q, m*128+p]
        mslice = np.ascontiguousarray(
            mask8_full[q0 : q0 + QS, :].T.reshape(ST, 128, QS)
            .transpose(1, 0, 2)
            .reshape(128, MASKW)
        )
        qT_u8 = (
            np.ascontiguousarray(query[q0 : q0 + QS].transpose(2, 1, 0))
            .astype(bf16)
            .view(np.uint8)
        )  # [128, B, 1024]
        blk0 = np.zeros([128, 5776], np.uint8)
        blk0[:, 0:656] = cpk
        blk0[:, 656:1680] = qT_u8[:, 0]
        blk0[:, 1680:5776] = key8_u8[:, 0, :4096]
        blkm = np.zeros([128, 12288], np.uint8)
        blkm[:, 0:4096] = key8_u8[:, 0, 4096:]
        blkm[:, 4096:] = val_u8[:, 0]
        blk = np.zeros([128, 3, 17408], np.uint8)
        for b in range(1, B):
            blk[:, b - 1, 0:1024] = qT_u8[:, b]
            blk[:, b - 1, 1024:9216] = key8_u8[:, b]
            blk[:, b - 1, 9216:17408] = val_u8[:, b]
        in_maps.append(
            {
                "blk0": blk0,
                "blkm": blkm,
                "blk": blk,
                "mask8": mslice,
                "vtag": np.zeros([KVER], np.float32),
            }
        )

    trace = bool(int(os.environ.get("KERNEL_TRACE", "0")))
    kw = {}
    if trace:
        kw = dict(trace=True, trace_cores=[0])
    try:
        res = run_bass_kernel_spmd(nc, in_maps, core_ids=list(range(NCORES)), **kw)
    except Exception:
        # transient device wedge (e.g. NRT_EXEC_UNIT_UNRECOVERABLE from an
        # earlier crashed process): one retry after the runtime re-opens
        res = run_bass_kernel_spmd(nc, in_maps, core_ids=list(range(NCORES)), **kw)
    LAST_RESULT = res
    # per-core out is [128, (b, qt, c)]; q_local = qt*128 + p
    cores = []
    for r in res.results:
        oc = r["out"].reshape(128, B, QT, C)
        cores.append(np.ascontiguousarray(oc.transpose(2, 0, 1, 3)).reshape(QS, B, C))
    out = np.concatenate(cores, axis=0)
    return out


# revision 36
# speedup vs baseline: 1.1780x; 1.1780x over previous
"""AttentionHead kernel for 8x TRN2 NeuronCores (Bass/Tile on Bacc).

Problem: single-head attention, S=4096, B=4, D=128, C=K=V=64, f32 inputs,
int32 {0,1} mask [1, S, S] applied before softmax (mask==0 -> -inf).

Sharding: queries sharded across 8 cores (512 q/core, all 4 batches per
core).

Math (per core, per batch), all PE contractions on partitions:
  qq = W2 q + b2 where W2 = wk^T wq (host-folded; per-q bias bk.q is
       softmax-invariant and dropped), cast fp8e4.
  scores^T[s, q] = sum_d key8[d,s] qq8[d,q]  +  maskbias[s, q]
       computed as ONE fp8 DoubleRow matmul per 128-s-tile chunk:
       lhsT [128, 2, 128] = [key8_tile | identity]   (host-interleaved)
       rhs  [128, 2, 512] = [qq8 | mask8_chunk]      (custom-stride AP over
            one SBUF arena [qq_b3..qq_b0 | mask(16K)]; the identity j-slot
            delivers maskbias = -240*(1-mask) exactly for free. The arena
            order makes each AP's bounding footprint cover only
            already-written qq slots and mask chunks <= m, so scores gate
            progressively on mask DMA arrival with no WAR hazards.)
  alpha = exp(scores/8)  (ACT, [128, 1536] groups, writes fp8 directly;
       masked entries exp(~-28) underflow to exactly 0)
  v_ext[s, c'] = fp8(value_tile^T wv), c'=64 column = 1 (memset), built
       directly in [s, c'] orientation (no transposes); bias bv deferred.
  comb[c', q] += v_ext_pair^T alpha_pair   (fp8 DoubleRow, K=256: two
       s-tiles per matmul; row 64 accumulates the softmax denominator)
  out[q, :] = comb[0:64]/comb[64] + bv     (PE transpose + fused
       affine_then_add: *recip + bv, bv host-replicated [128, 64])

Perf structure: ACT exp (11 instrs x ~1.55us per batch) is the bottleneck
engine; PE (scores 32 + va2 16 + vproj 32 + qq/epi per batch) runs ~50us
busy with slack, DVE ~15us, DMA ~8.5 MiB/core. Deep software pipeline:
group g scores || g-1 exp || lagged va2 pairs || staggered vproj/qq/epilogue
pending tasks keep every engine fed across batch seams.
"""

import os
import sys

import numpy as np

if "/opt/trn_rl_repo" not in sys.path:
    sys.path.insert(0, "/opt/trn_rl_repo")

S, B, D, C = 4096, 4, 128, 64
NCORES = 8
QS = S // NCORES  # 512 queries per core
QT = QS // 128  # 4 q tiles
ST = S // 128  # 32 s tiles per batch
NG = 11  # exp groups per batch: 10x(3 chunks) + 1x(2 chunks)
NP = ST // 2  # 16 va2 pairs per batch
SLOT = 128  # v_ext slot stride in elements (64 proj + 1 ones + pad;
# LDWEIGHTS DoubleRow requires well-aligned j-plane strides — 68 fails
# the walrus ISA check, 128 is the micro-proven shape)
MASKW = ST * QS  # 16384 arena mask columns
SCALE = 0.125  # 1/sqrt(64)
MASKBIAS = -240.0  # exact in fp8e4m3; exp(scale*(x-240)) == 0 for |x|<~100
ALPHA_FP8 = True  # False: bf16 alpha + non-DR va2 (higher precision)

LAST_RESULT = None
KVER = 52  # bumped per kernel revision: defeats HLO-fingerprint NEFF-cache aliasing


def _install_ntff_hook():
    """The grading/axon image lacks antenv.axon_hooks; recreate it so
    trace=True can capture NTFF profiles. Harmless no-op when unavailable."""
    import types

    try:
        import antenv

        try:
            from antenv import axon_hooks  # noqa: F401

            return
        except ImportError:
            pass
        from trn_agent_boot.trn_boot import _ntff_profile_via_ctypes

        mod = types.ModuleType("antenv.axon_hooks")
        _h = [_ntff_profile_via_ctypes("/opt/axon/libaxon_pjrt.so")]
        mod.get_axon_ntff_profile_hook = lambda: _h[0]
        mod.set_axon_ntff_profile_hook = lambda h: _h.__setitem__(0, h)
        sys.modules["antenv.axon_hooks"] = mod
        antenv.axon_hooks = mod
    except Exception:
        pass


def _gwidth(g):
    """chunks in group g (local index); group 0 is 2-wide so the seam
    iteration (next-batch pending flood) faces a short exp deadline"""
    return 2 if g == 0 else 3


def _gcol(g):
    """first chunk index of group g"""
    return 0 if g == 0 else 3 * g - 1


def _build_nc():
    import concourse.mybir as mybir
    from concourse import bacc
    from concourse.masks import make_identity
    from concourse.tile import TileContext

    f32 = mybir.dt.float32
    bf16 = mybir.dt.bfloat16
    f8 = mybir.dt.float8e4
    AF = mybir.ActivationFunctionType
    DR = mybir.MatmulPerfMode.DoubleRow
    a_dt = f8 if ALPHA_FP8 else bf16

    nc = bacc.Bacc("TRN2")

    # Inputs byte-packed into few large DMA blocks (each dma_start costs
    # ~700ns serial issue time on the SP sequencer):
    #   blk0: consts(656) | qT_b0 bf16(1024) | key8_b0 chunks 0-15 (4096)
    #   blkm: key8_b0 chunks 16-31 (4096) | valT_b0 bf16 (8192)
    #   blk[b-1] (b=1..3): qT(1024) | key8(8192) | valT(8192)
    # key8 slot layout per s-tile: [key_tile fp8 (128) | identity fp8 (128)]
    u8 = mybir.dt.uint8
    blk0_d = nc.dram_tensor("blk0", [128, 5776], u8, kind="ExternalInput")
    blkm_d = nc.dram_tensor("blkm", [128, 12288], u8, kind="ExternalInput")
    blk_d = nc.dram_tensor("blk", [128, 3, 17408], u8, kind="ExternalInput")
    # mask pre-swizzled on host to the arena layout [p, m*512+q]
    mask8_d = nc.dram_tensor("mask8", [128, MASKW], f8, kind="ExternalInput")
    # output layout [p, (b, qt, c)]: one contiguous 1KB-per-partition DMA
    # per batch (the [q, b, c] layout needed 256B descriptors); host
    # unpacks to [QS, B, C]
    out_d = nc.dram_tensor("out", [128, B * QT * C], f32, kind="ExternalOutput")
    # dummy input whose shape encodes the kernel revision: the PJRT-side NEFF
    # cache keys on the HLO signature (not the embedded BIR), so same-shaped
    # kernel revisions would otherwise silently alias to a stale executable.
    nc.dram_tensor("vtag", [KVER], f32, kind="ExternalInput")

    with TileContext(nc) as tc:
        with (
            tc.tile_pool(name="consts", bufs=1) as consts,
            tc.tile_pool(name="big", bufs=1) as big,
            tc.tile_pool(name="pb", bufs=2) as pb,
            tc.tile_pool(name="apool", bufs=2) as apool,
            tc.tile_pool(name="work", bufs=4) as work,
            tc.tile_pool(name="scps", bufs=2, space="PSUM") as scps,
            tc.tile_pool(name="ppps", bufs=1, space="PSUM") as ppps,
            tc.tile_pool(name="accps", bufs=1, space="PSUM") as accps,
        ):
            # ---------------- constants ----------------
            ident_f = consts.tile([128, 128], f32, tag="ident_f")
            make_identity(nc, ident_f[:])

            blk0 = big.tile([128, 5776], u8, tag="blk0")
            nc.sync.dma_start(out=blk0[:], in_=blk0_d[:, :])
            w2T = blk0[:, 0:256].bitcast(bf16)
            wvT = blk0[:, 256:384].bitcast(bf16)
            b2 = blk0[:, 384:388].bitcast(f32)
            bvrep = blk0[:, 400:656].bitcast(f32)

            # arena: [qq_b3 qq_b2 qq_b1 qq_b0 | mask (MASKW)] fp8.
            # qq slots REVERSED and ahead of the mask: scores(b, m)'s AP
            # bounding footprint is then [qq_b .. mask_m], i.e. only already-
            # written qq slots (no WAR on future batches' qq) and only mask
            # chunks <= m (progressive gating on mask DMA arrival).
            AQ = B * QS
            arena = big.tile([128, AQ + MASKW], f8, tag="arena")

            # One dma_start's descriptors spread across all 16 physical
            # queues, but each dma_start costs ~700ns of serial issue time on
            # its engine's sequencer. So: few, large dma_starts, spread across
            # engine sequencers (SP for startup-critical, idle Pool/DVE for
            # bulk prefetch) so issues proceed in parallel.
            def load_mask():
                for j in range(4):
                    nc.sync.dma_start(
                        out=arena[:, AQ + j * 4096 : AQ + (j + 1) * 4096],
                        in_=mask8_d[:, j * 4096 : (j + 1) * 4096],
                    )

            def scores_rhs(b, m):
                """custom AP [128, 2, 512]: j=0 -> qq_b, j=1 -> mask chunk m
                (pairs lhsT slot [key | I])"""
                o = (B - 1 - b) * QS
                base = arena[:, o : o + QS]
                ap = base.unsqueeze(1)
                l = ap.ap
                l[1] = [AQ + m * QS - o, 2]
                ap.ap = l
                return ap

            blk_by_b = {}

            def load_batch(b):
                blk = pb.tile([128, 17408], u8, tag="blk")
                nc.sync.dma_start(out=blk[:], in_=blk_d[:, b - 1, :])
                blk_by_b[b] = blk

            def qt_ap(b):
                if b == 0:
                    return blk0[:, 656:1680].bitcast(bf16)
                return blk_by_b[b][:, 0:1024].bitcast(bf16)

            def key_lhsT(b, m):
                if b == 0:
                    if m < 16:
                        sl = blk0[:, 1680 + m * 256 : 1680 + (m + 1) * 256]
                    else:
                        sl = blkm[:, (m - 16) * 256 : (m - 15) * 256]
                else:
                    sl = blk_by_b[b][:, 1024 + m * 256 : 1024 + (m + 1) * 256]
                return sl.bitcast(f8).rearrange("p (j s) -> p j s", j=2)

            def val_slice(b, st):
                if b == 0:
                    sl = blkm[:, 4096 + st * 256 : 4096 + (st + 1) * 256]
                else:
                    sl = blk_by_b[b][:, 9216 + st * 256 : 9216 + (st + 1) * 256]
                return sl.bitcast(bf16)

            def qq_tasks(b, qT):
                cell = {}

                def qq_mm():
                    qq_ps = ppps.tile([128, QS], f32, tag="pp", name="qq_ps")
                    nc.tensor.matmul(qq_ps[:], w2T, qT, start=True, stop=True)
                    cell["ps"] = qq_ps

                def qq_cp():
                    nc.vector.tensor_scalar_add(
                        out=arena[:, (B - 1 - b) * QS : (B - b) * QS],
                        in0=cell["ps"][:],
                        scalar1=b2,
                    )

                return [(qq_mm, qq_cp)]

            def v_tasks(vb, v_ext):
                """Direct-orientation vproj: out[s, c] tiles, batched copies."""
                pairs = []
                # ones column: c'=64 of each slot
                pairs.append(
                    (
                        lambda: nc.vector.memset(
                            v_ext[:].rearrange("p (t c) -> p t c", c=SLOT)[
                                :, :, C : C + 1
                            ],
                            1.0,
                        ),
                        None,
                    )
                )
                # first group small so its copy lands (in program order)
                # before the first va2 pair enters the PE queue
                bounds = [0, 2, 8, 14, 20, 26, 32]
                for gi in range(len(bounds) - 1):
                    g0, g1 = bounds[gi], bounds[gi + 1]
                    gs = g1 - g0
                    cell = {}
                    for k in range(gs):

                        def vp_mm(k=k, g0=g0, cell=cell, first=(k == 0)):
                            if first:
                                cell["ps"] = ppps.tile(
                                    [128, 7 * C], f32, tag="pp", name="vp_ps"
                                )
                            nc.tensor.matmul(
                                cell["ps"][:, k * C : (k + 1) * C],
                                val_slice(vb, g0 + k),
                                wvT,
                                start=True,
                                stop=True,
                            )

                        pairs.append((vp_mm, None))

                    def vp_cp(g0=g0, gs=gs, cell=cell):
                        nc.vector.tensor_copy(
                            out=v_ext[:, g0 * SLOT : (g0 + gs) * SLOT].rearrange(
                                "p (t c) -> p t c", c=SLOT
                            )[:, :, :C],
                            in_=cell["ps"][:, : gs * C].rearrange(
                                "p (t c) -> p t c", c=C
                            ),
                        )

                    pairs.append((None, vp_cp))
                return pairs

            def stagger(pairs):
                """Each step emits the PREVIOUS task's copy before this task's
                mm so the single-buffer pp ring never stalls the PE queue."""
                steps = []
                prev_cp = [None]

                def mk(mm, pc):
                    def step():
                        if pc is not None:
                            pc()
                        if mm is not None:
                            mm()

                    return step

                for mm, cp in pairs:
                    steps.append(mk(mm, prev_cp[0]))
                    prev_cp[0] = cp
                if prev_cp[0] is not None:
                    steps.append(lambda pc=prev_cp[0]: pc())
                return steps

            def epilogue_tasks(b, acc_ps, last=False):
                cell = {}
                steps = []

                def comb_step():
                    comb = work.tile([C + 1, QS], f32, tag="comb")
                    nc.vector.tensor_copy(out=comb[:], in_=acc_ps[:])
                    fin = work.tile([128, QT * C], f32, tag="fin")
                    cell["comb"] = comb
                    cell["fin"] = fin

                steps.append(comb_step)
                for qt in range(QT):

                    def qt_step(qt=qt):
                        if last:
                            # sc ring is free after the final exp; borrowing
                            # it unserializes the tail epilogue (pp bufs=1)
                            ot_ps = scps.tile(
                                [128, 1536], f32, tag="sc", name="sc"
                            )[:, : C + 1]
                        else:
                            ot_ps = ppps.tile(
                                [128, C + 1], f32, tag="pp", name="ot_ps"
                            )
                        nc.tensor.transpose(
                            ot_ps[:],
                            cell["comb"][:, qt * 128 : (qt + 1) * 128],
                            ident_f[: C + 1, : C + 1],
                        )
                        recip = work.tile([128, 1], f32, tag="recip")
                        nc.vector.reciprocal(recip[:], ot_ps[:, C : C + 1])
                        nc.vector.affine_then_add(
                            out=cell["fin"][:, qt * C : (qt + 1) * C],
                            in0=ot_ps[:, :C],
                            in1=bvrep,
                            scale=recip[:],
                            bias=0.0,
                        )

                    steps.append(qt_step)
                    if qt == 1:

                        def out_half():
                            nc.sync.dma_start(
                                out=out_d[:, b * QT * C : b * QT * C + 2 * C],
                                in_=cell["fin"][:, : 2 * C],
                            )

                        steps.append(out_half)

                def out_step():
                    nc.sync.dma_start(
                        out=out_d[:, b * QT * C + 2 * C : (b + 1) * QT * C],
                        in_=cell["fin"][:, 2 * C :],
                    )

                steps.append(out_step)
                return steps

            # ---------- flat pipeline over all B*NG groups ----------
            # Batch 0 prologue runs INLINE (not via pending): qq + all vproj
            # matmuls execute on PE while the mask arena streams in, instead
            # of scores(0,*) blocking the in-order PE queue on the mask DMAs.
            # batch-0 DMA order: blk0 (consts+qT+key8a, scores-critical),
            # mask (progressive), blkm (key8b+valT; vproj is pending-drained
            # so it never blocks the first scores in the PE queue)
            load_mask()
            blkm = big.tile([128, 12288], u8, tag="blkm")
            nc.sync.dma_start(out=blkm[:], in_=blkm_d[:, :])
            vext_by_b = {}
            alpha_by_b = {}
            for mm, cp in qq_tasks(0, qt_ap(0)):
                mm()
                if cp is not None:
                    cp()
            v_ext0 = pb.tile([128, ST * SLOT], a_dt, tag="v_ext")
            vext_by_b[0] = v_ext0
            pending = stagger(v_tasks(0, v_ext0))

            total = B * NG
            KEXP = 1  # exp lags scores by 1 group
            scs = {}
            acc_by_b = {}
            va_cursor = 0  # global va2 pair index
            exp_done = -1  # last global group exp'd

            def issue_va2(limit_pairs):
                """Issue ready va2 pairs up to global pair index `limit_pairs`
                (exclusive)."""
                nonlocal va_cursor
                while va_cursor < min(limit_pairs, B * NP):
                    v = va_cursor
                    b_v, u = divmod(v, NP)
                    if u == 0:
                        acc_by_b[b_v] = accps.tile(
                            [C + 1, QS], f32, tag="acc", name="acc"
                        )
                    acc = acc_by_b[b_v]
                    v_ext = vext_by_b[b_v]
                    alpha = alpha_by_b[b_v]
                    if ALPHA_FP8:
                        lhsT = v_ext[:, u * 2 * SLOT : (u + 1) * 2 * SLOT].rearrange(
                            "p (j c) -> p j c", c=SLOT
                        )[:, :, : C + 1]
                        rhs = alpha[:, u * 1024 : (u + 1) * 1024].rearrange(
                            "p (j q) -> p j q", j=2
                        )
                        nc.tensor.matmul(
                            acc[:],
                            lhsT,
                            rhs,
                            start=(u == 0),
                            stop=(u == NP - 1),
                            perf_mode=mybir.MatmulPerfMode.DoubleRow,
                        )
                    else:
                        for h in range(2):
                            st = 2 * u + h
                            nc.tensor.matmul(
                                acc[:],
                                v_ext[:, st * SLOT : st * SLOT + C + 1],
                                alpha[:, st * 512 : (st + 1) * 512],
                                start=(st == 0),
                                stop=(st == ST - 1),
                            )
                    va_cursor += 1
                    if u == NP - 1:
                        ep = epilogue_tasks(
                            b_v, acc_by_b.pop(b_v), last=(b_v == B - 1)
                        )
                        pending[:0] = [ep[0]]
                        pending.extend(ep[1:])
                        del vext_by_b[b_v]
                        del alpha_by_b[b_v]

            for gp in range(total + KEXP + 2):
                if gp < total:
                    b_s, g_s = divmod(gp, NG)
                    if g_s == 0:
                        alpha_by_b[b_s] = apool.tile(
                            [128, ST * 512], a_dt, tag="alpha", name="alpha"
                        )
                        if b_s + 1 < B:
                            load_batch(b_s + 1)
                            v_ext_n = pb.tile([128, ST * SLOT], a_dt, tag="v_ext")
                            vext_by_b[b_s + 1] = v_ext_n
                            pending.extend(
                                stagger(
                                    qq_tasks(b_s + 1, qt_ap(b_s + 1))
                                    + v_tasks(b_s + 1, v_ext_n)
                                )
                            )
                    w = _gwidth(g_s)
                    sc = scps.tile([128, 1536], f32, tag="sc", name="sc")
                    for ci in range(w):
                        m = _gcol(g_s) + ci
                        nc.tensor.matmul(
                            sc[:, ci * 512 : (ci + 1) * 512],
                            key_lhsT(b_s, m),
                            scores_rhs(b_s, m),
                            start=True,
                            stop=True,
                            perf_mode=DR,
                        )
                    scs[gp] = (sc, b_s, g_s, w)
                if KEXP <= gp < total + KEXP:
                    v = gp - KEXP
                    sc, b_e, g_e, w = scs.pop(v)
                    alpha = alpha_by_b[b_e]
                    c0 = _gcol(g_e) * 512
                    nc.scalar.activation(
                        out=alpha[:, c0 : c0 + w * 512],
                        in_=sc[:, : w * 512],
                        func=AF.Exp,
                        scale=SCALE,
                    )
                    exp_done = v
                    # va2 pairs fully covered by exp'd groups (one group slack)
                    eb, eg = divmod(exp_done, NG)
                    # within batch eb: chunks done through _gcol(eg)+_gwidth(eg)
                    chunks_done = _gcol(eg) + _gwidth(eg)
                    pairs_done = chunks_done // 2
                    issue_va2(eb * NP + max(0, pairs_done - 2))
                if gp >= total + KEXP:
                    issue_va2(B * NP)
                for thresh in (0, 0, 6, 12, 20):
                    if len(pending) > thresh:
                        pending.pop(0)()
            issue_va2(B * NP)
            while pending:
                pending.pop(0)()

    nc.finalize()
    return nc


_nc_cache = None


def kernel(**inputs):
    global _nc_cache, LAST_RESULT
    _install_ntff_hook()
    import ml_dtypes

    from concourse.bass_utils import run_bass_kernel_spmd

    bf16 = ml_dtypes.bfloat16
    f8 = ml_dtypes.float8_e4m3
    arrs = {k: np.asarray(v) for k, v in inputs.items()}

    keyT = np.ascontiguousarray(
        arrs["key"].astype(np.float32).transpose(2, 1, 0)
    )  # [D, B, S]
    key8 = np.zeros([D, B, ST * 256], dtype=f8)
    eye8 = np.eye(128, dtype=np.float32).astype(f8)
    k8 = keyT.astype(f8)
    for st in range(ST):
        key8[:, :, st * 256 : st * 256 + 128] = k8[:, :, st * 128 : (st + 1) * 128]
        key8[:, :, st * 256 + 128 : (st + 1) * 256] = eye8[:, None, :]

    value = np.ascontiguousarray(
        arrs["value"].astype(np.float32).transpose(2, 1, 0)
    ).astype(bf16)  # [D, B, S]
    query = np.ascontiguousarray(arrs["query"], dtype=np.float32)
    mask = np.ascontiguousarray(arrs["mask"], dtype=np.int32)
    if mask.ndim == 3:
        mask = mask[0]
    maskbias = np.where(mask == 0, np.float32(MASKBIAS), np.float32(0.0))
    mask8_full = maskbias.astype(f8)  # [S(q-axis? no: rows=q of reference), S]

    wk_f = arrs["wk_w"].astype(np.float64)
    wq_f = arrs["wq_w"].astype(np.float64)
    # scores = (wk key).(wq q + bq) = key^T (W2 q + b2); lhsT for the qq
    # matmul is W2^T = wq^T wk
    w2T = np.ascontiguousarray(wq_f.T @ wk_f).astype(np.float32).astype(bf16)
    b2h = np.ascontiguousarray(
        (wk_f.T @ arrs["wq_b"].astype(np.float64)).astype(np.float32)
    ).reshape(D, 1)
    wvT = np.ascontiguousarray(arrs["wv_w"].astype(np.float32).T).astype(bf16)
    bvrep = np.ascontiguousarray(
        np.broadcast_to(arrs["wv_b"].astype(np.float32)[None, :], (128, C))
    ).astype(np.float32)
    cpk = np.zeros([128, 656], np.uint8)
    cpk[:, 0:256] = w2T.view(np.uint8).reshape(128, 256)
    cpk[:, 256:384] = wvT.view(np.uint8).reshape(128, 128)
    cpk[:, 384:388] = b2h.view(np.uint8).reshape(128, 4)
    cpk[:, 400:656] = bvrep.view(np.uint8).reshape(128, 256)

    key8_u8 = key8.view(np.uint8)  # [128, B, 8192]
    val_u8 = value.view(np.uint8)  # [128, B, 8192]

    if _nc_cache is None:
        _nc_cache = _build_nc()
    nc = _nc_cache

    in_maps = []
    for i in range(NCORES):
        q0 = i * QS
        # scores^T[s, q] is masked by mask[q_global, s]; arena layout
        # [p, m*512 + q] = maskbias[q0+q, m*128+p]
        mslice = np.ascontiguousarray(
            mask8_full[q0 : q0 + QS, :].T.reshape(ST, 128, QS)
            .transpose(1, 0, 2)
            .reshape(128, MASKW)
        )
        qT_u8 = (
            np.ascontiguousarray(query[q0 : q0 + QS].transpose(2, 1, 0))
            .astype(bf16)
            .view(np.uint8)
        )  # [128, B, 1024]
        blk0 = np.zeros([128, 5776], np.uint8)
        blk0[:, 0:656] = cpk
        blk0[:, 656:1680] = qT_u8[:, 0]
        blk0[:, 1680:5776] = key8_u8[:, 0, :4096]
        blkm = np.zeros([128, 12288], np.uint8)
        blkm[:, 0:4096] = key8_u8[:, 0, 4096:]
        blkm[:, 4096:] = val_u8[:, 0]
        blk = np.zeros([128, 3, 17408], np.uint8)
        for b in range(1, B):
            blk[:, b - 1, 0:1024] = qT_u8[:, b]
            blk[:, b - 1, 1024:9216] = key8_u8[:, b]
            blk[:, b - 1, 9216:17408] = val_u8[:, b]
        in_maps.append(
            {
                "blk0": blk0,
                "blkm": blkm,
                "blk": blk,
                "mask8": mslice,
                "vtag": np.zeros([KVER], np.float32),
            }
        )

    trace = bool(int(os.environ.get("KERNEL_TRACE", "0")))
    kw = {}
    if trace:
        kw = dict(trace=True, trace_cores=[0])
    try:
        res = run_bass_kernel_spmd(nc, in_maps, core_ids=list(range(NCORES)), **kw)
    except Exception:
        # transient device wedge (e.g. NRT_EXEC_UNIT_UNRECOVERABLE from an
        # earlier crashed process): one retry after the runtime re-opens
        res = run_bass_kernel_spmd(nc, in_maps, core_ids=list(range(NCORES)), **kw)
    LAST_RESULT = res
    # per-core out is [128, (b, qt, c)]; q_local = qt*128 + p
    cores = []
    for r in res.results:
        oc = r["out"].reshape(128, B, QT, C)
        cores.append(np.ascontiguousarray(oc.transpose(2, 0, 1, 3)).reshape(QS, B, C))
    out = np.concatenate(cores, axis=0)
    return out
